# revision 13
# baseline (speedup 1.0000x reference)
"""Trainium2 Bass kernel for CFConv (gnn_message_passing).

out[n] = in_node_feat[n] * sum_{e: tgt(e)=n} filt(d_e), where filt(d) is a
function of the scalar edge distance only. The device builds a 64-point
filter table T[b] = filt(b*h) plus precombined derivative tables so the
quadratic-Lagrange interpolation f(u) = T0[j] + t*T1[j] + t^2*T2[j]
(u = d/h, j = int(u), t = u - j) factors into three per-(node, bucket)
moment histograms accumulated on the tensor engine, followed by 3 table
matmuls per 256 nodes.

Edges are host-classified by bucket: common (j <= 39, ~99.7%) vs rare
(j <= 61); nodes owning rare edges are packed into two dedicated
supergroups so common chunks build only 39 one-hot rows on DVE instead
of 64. Groups hold 4 nodes (~1 tile of 128 edges each); the histogram
is one matmul per tile into 16-col padded PSUM blocks. Supergroups are
paired into PSUM partitions 0:64 / 64:128 so a single [128, 768] Act
copy evacuates two supergroups and one [128, 512] PSUM tile holds both
table-matmul outputs. GPSIMD cannot touch PSUM, so PSUM reads stay on
Act/DVE; the feature modulation runs fused on DVE (PSUM x bf16) for
some pairs and Act-copy + DVE-4x-multiply for the rest (engine
balance). Positions move as fp16, mask/features/outputs as bf16.

Nodes are sharded 8 ways (6250/core); each core processes exactly the
edges targeting its nodes, so no collective is needed. Host prep does
only index work (shard, classify, group, pad, layout); distances,
table, histograms, reduction and modulation all run on device.
"""

import sys
import numpy as np

sys.path.insert(0, "/opt/trn_rl_repo")

N = 50000
OUT_DIM = 128
NF = 64
HID = 64
NCORES = 8
NODES_PC = N // NCORES           # 6250
GROUP = 4                        # nodes per window group
SG = 64                          # groups per supergroup (256 nodes)
NSG = 26
NGROUPS = SG * NSG               # 1664
NODES_PC_PAD = NGROUPS * GROUP   # 6656
SGN = SG * GROUP                 # nodes per supergroup (256)
NB = 64                          # table buckets
NM = 3                           # moments 1, t, t^2
DMAX = 8.5                       # table covers d in [0, DMAX]
SCALE = (NB - 1) / DMAX
P = 128

UT_HOST = 39.4                   # host: edges with u > UT_HOST are rare
NB_C = 39                        # common one-hot rows: j in 1..39
NB_R = 61                        # rare one-hot rows: j in 1..61
CLAMP_C = 39.49
CLAMP_R = 61.49
CLAMP_LO = 1.001

RARE_SGS = (1, 2)                # supergroups holding rare nodes
NCH = 4                          # sg0 fill chunk + 3 balanced chunks
MOD_ACT_PAIRS = 13               # pairs modulated via Act-copy + DVE mult

_cache = {}


def _lpt_pack(node_ids, deg, ngroups):
    """Pack node_ids into ngroups groups of <= GROUP nodes, balancing total
    degree (greedy LPT). Returns ([ngroups, GROUP] ids with -1 pad, sums)."""
    import heapq
    node_ids = np.asarray(node_ids, np.int64)
    order = node_ids[np.argsort(-deg[node_ids], kind="stable")]
    heap = [(0, g, 0) for g in range(ngroups)]  # (sum, group, count)
    heapq.heapify(heap)
    groups = -np.ones((ngroups, GROUP), np.int64)
    gsum = np.zeros(ngroups, np.int64)
    deferred = []
    for n in order:
        while True:
            s, g, cnt = heapq.heappop(heap)
            if cnt < GROUP:
                break
            deferred.append((s, g, cnt))
        groups[g, cnt] = n
        gsum[g] = s + int(deg[n])
        heapq.heappush(heap, (gsum[g], g, cnt + 1))
        for item in deferred:
            heapq.heappush(heap, item)
        deferred.clear()
    return groups, gsum


def _host_prep(inputs):
    import ml_dtypes

    pos = np.asarray(inputs["node_pos"], dtype=np.float32)
    ei = np.asarray(inputs["edge_index"])
    src = ei[0].astype(np.int64)
    tgt = ei[1].astype(np.int64)

    core = tgt // NODES_PC
    ln_all = tgt - core * NODES_PC

    # classify edges by host-side bucket coordinate
    diff = pos[src] - pos[tgt]
    u_all = np.sqrt((diff * diff).sum(axis=1)) * np.float32(SCALE)

    NRG = SG * len(RARE_SGS)     # rare groups (128)
    NCG = NGROUPS - NRG          # common groups
    per_core = []
    sizes_all = np.zeros((NCORES, NGROUPS), np.int64)
    for c in range(NCORES):
        idx = np.nonzero(core == c)[0]
        ln = ln_all[idx]
        deg = np.bincount(ln, minlength=NODES_PC)
        rare_nodes = np.unique(ln[u_all[idx] > UT_HOST])
        assert len(rare_nodes) <= NRG * GROUP, len(rare_nodes)
        is_rare = np.zeros(NODES_PC, bool)
        is_rare[rare_nodes] = True
        common_nodes = np.nonzero(~is_rare)[0]

        gR, sR = _lpt_pack(rare_nodes, deg, NRG)
        gC, sC = _lpt_pack(common_nodes, deg, NCG)
        rR = np.argsort(-sR, kind="stable")
        rC = np.argsort(-sC, kind="stable")
        # order: [64 smallest commons (sg0)] [rare (sg1, sg2)] [commons desc]
        groups = np.concatenate(
            [gC[rC[-SG:]], gR[rR], gC[rC[:-SG]]], axis=0)
        gsum = np.concatenate([sC[rC[-SG:]], sR[rR], sC[rC[:-SG]]])
        sizes_all[c] = gsum
        n2g = np.zeros(NODES_PC, np.int64)
        n2s = np.zeros(NODES_PC, np.int64)
        valid = groups >= 0
        n2g[groups[valid]] = np.repeat(np.arange(NGROUPS), GROUP)[valid.ravel()]
        n2s[groups[valid]] = np.tile(np.arange(GROUP), NGROUPS)[valid.ravel()]
        per_core.append((idx, ln, n2g, n2s, groups))

    tiles_g = np.maximum(1, (sizes_all.max(axis=0) + P - 1) // P)
    gbase = np.zeros(NGROUPS, np.int64)
    np.cumsum(tiles_g[:-1], out=gbase[1:])
    T_TILES = int(tiles_g.sum())
    E_pad = T_TILES * P

    feats = np.asarray(inputs["in_node_feat"], dtype=np.float32)
    in_maps = []
    node_of = []                # per core: output column -> local node (-1 pad)
    for c in range(NCORES):
        idx, ln, n2g, n2s, groups = per_core[c]
        g = n2g[ln]
        slot_in_g = n2s[ln]
        order = np.argsort(g, kind="stable")
        idx = idx[order]
        g = g[order]
        slot_in_g = slot_in_g[order]
        sizes = np.bincount(g, minlength=NGROUPS)
        starts = np.zeros(NGROUPS, np.int64)
        np.cumsum(sizes[:-1], out=starts[1:])
        within = np.arange(len(idx)) - starts[g]
        slot = gbase[g] * P + within

        def plane(vals):
            a = np.zeros(E_pad, np.float32)
            a[slot] = vals
            return np.ascontiguousarray(a.reshape(T_TILES, P).T)

        s_i, t_i = src[idx], tgt[idx]
        m = {}
        # source planes 0:3, target planes 3:6 (contiguous slices on device)
        m["posP"] = np.ascontiguousarray(np.concatenate(
            [plane(pos[s_i, 0]), plane(pos[s_i, 1]), plane(pos[s_i, 2]),
             plane(pos[t_i, 0]), plane(pos[t_i, 1]), plane(pos[t_i, 2])],
            axis=1)).astype(np.float16)

        # mask in group-major layout [P, GROUP, T_TILES]
        msk = np.zeros((E_pad, GROUP), np.float32)
        msk[slot, slot_in_g] = 1.0
        msk = msk.reshape(T_TILES, P, GROUP).transpose(1, 2, 0).reshape(
            P, GROUP * T_TILES)
        m["maskS"] = np.ascontiguousarray(msk).astype(ml_dtypes.bfloat16)

        # feature columns permuted into group order
        colnode = groups.reshape(-1)                    # [NODES_PC_PAD]
        f = np.zeros((P, NODES_PC_PAD), np.float32)
        valid = colnode >= 0
        f[:, valid] = feats[c * NODES_PC + colnode[valid]].T
        m["featT"] = np.ascontiguousarray(f).astype(ml_dtypes.bfloat16)
        node_of.append(colnode)
        in_maps.append(m)

    lo = float(np.asarray(inputs["lower_bound"]))
    hi = float(np.asarray(inputs["upper_bound"]))
    gamma = float(np.asarray(inputs["gamma"]))
    mu = np.linspace(lo, hi, NF, dtype=np.float32)
    W1 = np.asarray(inputs["W1"], dtype=np.float32)
    W2 = np.asarray(inputs["W2"], dtype=np.float32)
    b1 = np.asarray(inputs["b1"], dtype=np.float32)
    b2 = np.asarray(inputs["b2"], dtype=np.float32)
    # packed constants [P, 577]: ident | W1T | W2T | b1r | b2r | mur | gridc
    constP = np.zeros((P, 577), np.float32)
    constP[:, 0:128] = np.eye(P, dtype=np.float32)
    constP[0:NF, 128:192] = W1.T
    constP[0:HID, 192:320] = W2.T
    constP[:, 320:384] = b1[None, :]
    constP[:, 384:512] = b2[None, :]
    constP[:, 512:576] = mu[None, :]
    constP[:, 576] = np.arange(P, dtype=np.float32) / SCALE
    for m in in_maps:
        m["constP"] = constP
    return in_maps, tuple(int(x) for x in tiles_g), gamma, node_of


def _build(tiles_g, gamma):
    from concourse import bacc, mybir
    from concourse.tile import TileContext

    f32 = mybir.dt.float32
    f32r = mybir.dt.float32r
    f16 = mybir.dt.float16
    i32 = mybir.dt.int32
    bf16 = mybir.dt.bfloat16
    AF = mybir.ActivationFunctionType
    OP = mybir.AluOpType
    LN2 = float(np.log(2.0))

    tiles_g = np.asarray(tiles_g, np.int64)
    T_TILES = int(tiles_g.sum())
    gb = np.zeros(NGROUPS + 1, np.int64)
    np.cumsum(tiles_g, out=gb[1:])
    sgt = [int(gb[SG * s]) for s in range(NSG + 1)]
    # chunk 0 = sg0 (pipeline fill); sgs 1..25 split into 3 chunks with
    # balanced tile widths, chunk 1 keeping both rare sgs
    target = (T_TILES - sgt[1]) / 3.0
    chunk_sg = [0, 1]
    for c in range(2):
        lo = chunk_sg[-1]
        cand = range(max(lo + 1, 3), NSG - (1 - c))
        chunk_sg.append(min(
            cand, key=lambda s: abs((sgt[s] - sgt[lo]) - target)))
    chunk_sg.append(NSG)
    cb = [sgt[chunk_sg[c]] for c in range(NCH + 1)]
    T_CH = max(cb[i + 1] - cb[i] for i in range(NCH))
    T_R = sgt[3] - sgt[1]            # rare supergroup tiles (sgs 1, 2)

    # tile -> (sg-local group, k, lastk)
    tinfo = []
    for g in range(NGROUPS):
        cnt = int(tiles_g[g])
        for k in range(cnt):
            tinfo.append((g % SG, k, cnt - 1))

    nc = bacc.Bacc("TRN2", target_bir_lowering=False, debug=False,
                   num_devices=NCORES)

    def din(name, shape, dt=f32):
        return nc.dram_tensor(name, shape, dt, kind="ExternalInput").ap()

    posP = din("posP", [P, 6 * T_TILES], f16)
    maskS = din("maskS", [P, GROUP * T_TILES], bf16)
    featT = din("featT", [P, NODES_PC_PAD], bf16)
    constP = din("constP", [P, 577])

    outT = nc.dram_tensor("outT", [P, NODES_PC_PAD], bf16,
                          kind="ExternalOutput").ap()

    with TileContext(nc) as tc:
        with (
            tc.tile_pool(name="const", bufs=1) as const,
            tc.tile_pool(name="chpos", bufs=1) as chpos,
            tc.tile_pool(name="chw1", bufs=1) as chw1,
            tc.tile_pool(name="chwork", bufs=2) as chwork,
            tc.tile_pool(name="chlb", bufs=2) as chlb,
            tc.tile_pool(name="chlbr", bufs=1) as chlbr,
            tc.tile_pool(name="chsc", bufs=2) as chsc,
            tc.tile_pool(name="sgt", bufs=2) as sgtp,
            tc.tile_pool(name="outs", bufs=2) as outsp,
            tc.tile_pool(name="pstap", bufs=2, space="PSUM") as pstap,
            tc.tile_pool(name="psout", bufs=2, space="PSUM") as psout,
        ):
            posv = posP.rearrange("p (k t) -> p k t", k=6)
            maskv = maskS.rearrange("p (g t) -> p g t", g=GROUP)

            def fetch_chunk(c):
                """Allocate chunk tiles and start their DMAs (pos + mask
                straight into the scat m0 slot)."""
                c0, c1 = cb[c], cb[c + 1]
                tl = c1 - c0
                post = chpos.tile([P, 6 * T_CH], f16, tag="post")
                pov = post[:].rearrange("p (k t) -> p k t", k=6)
                nc.sync.dma_start(out=pov[:, :, :tl], in_=posv[:, :, c0:c1])
                scat = chsc.tile([P, NM * GROUP * T_CH], bf16, tag="scat")
                scv = scat[:].rearrange("p (m g t) -> p m g t", m=NM,
                                        g=GROUP)
                nc.sync.dma_start(out=scv[:, 0, :, :tl],
                                  in_=maskv[:, :, c0:c1])
                return pov, scv

            fetched = {0: fetch_chunk(0)}

            # ---------- constants (one packed DMA) ----------
            cp = const.tile([P, 577], f32, tag="cp")
            nc.sync.dma_start(out=cp[:], in_=constP)
            id_s = cp[:, 0:128]
            W1T_s = cp[0:NF, 128:192]
            W2T_s = cp[0:HID, 192:320]
            b1_s = cp[:, 320:384]
            b2_s = cp[:, 384:512]
            mu_s = cp[:, 512:576]
            gr_s = cp[:, 576:577]

            ln2n = const.tile([P, 1], f32, tag="ln2n")
            nc.vector.memset(ln2n[:], -LN2)
            halfc = const.tile([P, 1], f32, tag="halfc")
            nc.vector.memset(halfc[:], 0.5)

            featT_s = const.tile([P, NODES_PC_PAD], bf16, tag="feat")

            # ---------- filter table ----------
            # TA[m] row k holds T_m[k+1] at partitions 0:61 and again at
            # 64:125 (lhsT for even/odd-slot supergroups).
            TA = []

            def emit_table():
                tg1 = const.tile([P, NF], f32, tag="tg1")
                tg2 = const.tile([P, NF], f32, tag="tg2")
                nc.vector.tensor_tensor(out=tg1[:],
                                        in0=gr_s.to_broadcast([P, NF]),
                                        in1=mu_s, op=OP.subtract)
                nc.scalar.activation(out=tg2[:], in_=tg1[:], func=AF.Square)
                rbf = const.tile([P, NF], f32, tag="rbf")
                nc.scalar.activation(out=rbf[:], in_=tg2[:], func=AF.Exp,
                                     scale=-gamma)

                ptr1 = psout.tile([NF, P], f32, tag="pro")
                nc.tensor.transpose(out=ptr1[:], in_=rbf[:], identity=id_s)
                x0t = const.tile([NF, P], f32, tag="x0t")
                nc.vector.tensor_copy(out=x0t[:], in_=ptr1[:])

                ph = psout.tile([P, HID], f32, tag="pro")
                nc.tensor.matmul(out=ph[:], lhsT=x0t[:], rhs=W1T_s,
                                 start=True, stop=True)
                pre1 = const.tile([P, HID], f32, tag="pre1")
                nc.vector.tensor_tensor(out=pre1[:], in0=ph[:], in1=b1_s,
                                        op=OP.add)
                e1 = const.tile([P, HID], f32, tag="e1")
                nc.scalar.activation(out=e1[:], in_=pre1[:], func=AF.Exp,
                                     bias=ln2n[:])
                x1 = const.tile([P, HID], f32, tag="x1")
                nc.scalar.activation(out=x1[:], in_=e1[:], func=AF.Ln,
                                     bias=halfc[:])

                ptr2 = psout.tile([HID, P], f32, tag="pro")
                nc.tensor.transpose(out=ptr2[:], in_=x1[:], identity=id_s)
                x1t = const.tile([HID, P], f32, tag="x1t")
                nc.vector.tensor_copy(out=x1t[:], in_=ptr2[:])

                pf = psout.tile([P, OUT_DIM], f32, tag="pro")
                nc.tensor.matmul(out=pf[:], lhsT=x1t[:], rhs=W2T_s,
                                 start=True, stop=True)
                pre2 = const.tile([P, OUT_DIM], f32, tag="pre2")
                nc.vector.tensor_tensor(out=pre2[:], in0=pf[:], in1=b2_s,
                                        op=OP.add)
                e2 = const.tile([P, OUT_DIM], f32, tag="e2")
                nc.scalar.activation(out=e2[:], in_=pre2[:], func=AF.Exp,
                                     bias=ln2n[:])
                Tf = const.tile([P, OUT_DIM], f32r, tag="Tf")
                nc.scalar.activation(out=Tf[:], in_=e2[:], func=AF.Ln,
                                     bias=halfc[:])

                # shifted copies on bucket rows 0..NB-1
                Tp_s = const.tile([NB, OUT_DIM], f32r, tag="tp")
                Tm_s = const.tile([NB, OUT_DIM], f32r, tag="tm")
                nc.sync.dma_start(out=Tp_s[:], in_=Tf[1:NB + 1, :])
                # row 0 pairs only with bucket 0, which never fires (u >= 1)
                nc.sync.dma_start(out=Tm_s[0:1, :], in_=Tf[0:1, :])
                nc.sync.dma_start(out=Tm_s[1:NB, :], in_=Tf[0:NB - 1, :])

                # T1 = (Tp - Tm)/2 ; T2 = (Tp + Tm)/2 - T
                T1b = const.tile([P, OUT_DIM], f32r, tag="t1b")
                T2b = const.tile([P, OUT_DIM], f32r, tag="t2b")
                ttmp = const.tile([NB, OUT_DIM], f32r, tag="ttmp")
                nc.vector.tensor_tensor(out=ttmp[:], in0=Tp_s[:],
                                        in1=Tm_s[:], op=OP.subtract)
                nc.vector.tensor_scalar(out=T1b[0:NB, :], in0=ttmp[:],
                                        scalar1=0.5, scalar2=None,
                                        op0=OP.mult)
                nc.vector.tensor_tensor(out=ttmp[:], in0=Tp_s[:],
                                        in1=Tm_s[:], op=OP.add)
                nc.vector.tensor_scalar(out=ttmp[:], in0=ttmp[:],
                                        scalar1=0.5, scalar2=None,
                                        op0=OP.mult)
                nc.vector.tensor_tensor(out=T2b[0:NB, :], in0=ttmp[:],
                                        in1=Tf[0:NB, :], op=OP.subtract)

                # bucket-(k+1)-at-row-k copies, duplicated at partition 64
                for m, srcT in enumerate((Tf, T1b, T2b)):
                    ta = const.tile([P, OUT_DIM], f32r, tag=f"ta{m}")
                    nc.sync.dma_start(out=ta[0:NB_R, :],
                                      in_=srcT[1:NB_R + 1, :])
                    nc.sync.dma_start(out=ta[64:64 + NB_R, :],
                                      in_=srcT[1:NB_R + 1, :])
                    TA.append(ta)

            lbR_v = [None]
            pair_state = {}

            # Pre-zero both tap buffers once: the 39/61-row histogram
            # matmuls never touch the pad partitions the pair evacuation
            # copy reads.
            tap_tiles = []
            for _ in range(2):
                t = pstap.tile([P, SG * 4 * GROUP], f32, tag="tap")
                nc.vector.memset(t[:], 0.0)
                tap_tiles.append(t)

            def consume_sg(s, lbv_c, lc0, scv_c, sc0):
                base = 64 if s % 2 else 0
                rows = NB_R if s in RARE_SGS else NB_C
                if s % 2 == 0:
                    tap = tap_tiles[(s // 2) % 2]
                    outPair = psout.tile([P, 2 * SGN], f32, tag="outP")
                    pair_state[0] = (tap, outPair)
                tap, outPair = pair_state[0]
                tapv = tap[:].rearrange("p (g m q) -> p g m q", m=4, q=GROUP)
                for tt in range(sgt[s], sgt[s + 1]):
                    gl, k, lastk = tinfo[tt]
                    if s in RARE_SGS:
                        lhsT = lbR_v[0][:, :, tt - sgt[RARE_SGS[0]]]
                    else:
                        lhsT = lbv_c[:, :, tt - lc0]
                    nc.tensor.matmul(
                        out=tapv[base:base + rows, gl, 0:NM, :],
                        lhsT=lhsT,
                        rhs=scv_c[:, :, :, tt - sc0],
                        start=(k == 0), stop=(k == lastk))
                if s % 2 == 0:
                    return
                # ---- pair complete: evac, tables, modulate, store ----
                pidx = s // 2
                tsb = sgtp.tile([P, SG * NM * GROUP], f32r, tag="tsb")
                tsbv = tsb[:].rearrange("p (g m q) -> p g m q", m=NM,
                                        q=GROUP)
                nc.scalar.copy(out=tsbv[:, :, :, :], in_=tapv[:, :, 0:NM, :])
                for sp in (s - 1, s):
                    b2_ = 64 if sp % 2 else 0
                    r2 = NB_R if sp in RARE_SGS else NB_C
                    o0 = (sp % 2) * SGN
                    for m in range(NM):
                        nc.tensor.matmul(
                            out=outPair[:, o0:o0 + SGN],
                            lhsT=TA[m][b2_:b2_ + r2, :],
                            rhs=tsbv[b2_:b2_ + r2, :, m, :],
                            start=(m == 0), stop=(m == NM - 1))
                outS = outsp.tile([P, 2 * SGN], bf16, tag="outS")
                fsl = featT_s[:, (s - 1) * SGN:(s + 1) * SGN]
                if pidx < MOD_ACT_PAIRS:
                    nc.scalar.copy(out=outS[:], in_=outPair[:])
                    nc.vector.tensor_tensor(out=outS[:], in0=outS[:],
                                            in1=fsl, op=OP.mult)
                else:
                    nc.vector.tensor_tensor(out=outS[:], in0=outPair[:],
                                            in1=fsl, op=OP.mult)
                nc.sync.dma_start(
                    out=outT[:, (s - 1) * SGN:(s + 1) * SGN], in_=outS[:])

            for c in range(NCH):
                c0, c1 = cb[c], cb[c + 1]
                tl = c1 - c0
                rlen = T_R if c == 1 else 0   # rare tiles lead chunk 1

                # ---------- chunk build ----------
                pov, scv = fetched.pop(c)

                wd = chw1.tile([P, 3 * T_CH], f16, tag="wd")
                wdv = wd[:].rearrange("p (a t) -> p a t", a=3)
                # diff (DVE fp16 2x), square in place (one Act op), sum
                # in fp16 on Pool (d^2 tolerates fp16: rel ~5e-4)
                nc.vector.tensor_tensor(
                    out=wdv[:, :, :tl], in0=pov[:, 0:3, :tl],
                    in1=pov[:, 3:6, :tl], op=OP.subtract)
                nc.scalar.activation(out=wdv[:, :, :tl], in_=wdv[:, :, :tl],
                                     func=AF.Square)
                nc.gpsimd.tensor_tensor(out=wdv[:, 0, :tl],
                                        in0=wdv[:, 0, :tl],
                                        in1=wdv[:, 1, :tl], op=OP.add)
                nc.gpsimd.tensor_tensor(out=wdv[:, 0, :tl],
                                        in0=wdv[:, 0, :tl],
                                        in1=wdv[:, 2, :tl], op=OP.add)
                # u = sqrt(d2)*SCALE, clamped so taps stay in range
                w0tile = chw1.tile([P, T_CH], f32, tag="w0")
                w0 = w0tile[:]
                nc.scalar.activation(out=w0[:, :tl], in_=wdv[:, 0, :tl],
                                     func=AF.Sqrt, scale=SCALE * SCALE)
                if rlen:
                    nc.gpsimd.tensor_scalar(
                        out=w0[:, :rlen], in0=w0[:, :rlen],
                        scalar1=CLAMP_LO, scalar2=CLAMP_R,
                        op0=OP.max, op1=OP.min)
                nc.gpsimd.tensor_scalar(
                    out=w0[:, rlen:tl], in0=w0[:, rlen:tl],
                    scalar1=CLAMP_LO, scalar2=CLAMP_C,
                    op0=OP.max, op1=OP.min)
                ji = chw1.tile([P, T_CH], i32, tag="ji")
                nc.scalar.copy(out=ji[:, :tl], in_=w0[:, :tl])
                jb = chwork.tile([P, T_CH], bf16, tag="jb")
                nc.gpsimd.tensor_copy(out=jb[:, :tl], in_=ji[:, :tl])
                tb = chwork.tile([P, T_CH], bf16, tag="tb")
                nc.gpsimd.tensor_tensor(out=tb[:, :tl], in0=w0[:, :tl],
                                        in1=jb[:, :tl], op=OP.subtract)

                # one-hot rows (4x DVE mode: all-SBUF packed bf16)
                lb = chlb.tile([P, NB_C * T_CH], bf16, tag="lb")
                lbv = lb[:].rearrange("p (b t) -> p b t", b=NB_C)
                for b in range(1, NB_C + 1):
                    nc.vector.tensor_scalar(out=lbv[:, b - 1, rlen:tl],
                                            in0=jb[:, rlen:tl],
                                            scalar1=float(b), scalar2=None,
                                            op0=OP.is_equal)
                if rlen:
                    lbr = chlbr.tile([P, NB_R * T_R], bf16, tag="lbr")
                    lbrv = lbr[:].rearrange("p (b t) -> p b t", b=NB_R)
                    for b in range(1, NB_R + 1):
                        nc.vector.tensor_scalar(out=lbrv[:, b - 1, :],
                                                in0=jb[:, :rlen],
                                                scalar1=float(b),
                                                scalar2=None,
                                                op0=OP.is_equal)
                    lbR_v[0] = lbrv

                # moment planes: m1 = mask*t, m2 = m1*t  (4x DVE mode)
                tbv = tb[:, :tl].rearrange("p (o t) -> p o t", o=1)
                nc.vector.tensor_tensor(
                    out=scv[:, 1, :, :tl], in0=scv[:, 0, :, :tl],
                    in1=tbv.to_broadcast([P, GROUP, tl]), op=OP.mult)
                nc.vector.tensor_tensor(
                    out=scv[:, 2, :, :tl], in0=scv[:, 1, :, :tl],
                    in1=tbv.to_broadcast([P, GROUP, tl]), op=OP.mult)

                # prefetch the next chunk before the consume-phase DMAs so
                # its transfers are not head-of-line blocked in the SP queue
                if c + 1 < NCH:
                    fetched[c + 1] = fetch_chunk(c + 1)
                if c == 0:
                    nc.sync.dma_start(out=featT_s[:], in_=featT)
                if c == 1:
                    # Emitted here so the table's Act ops and table-set
                    # loads queue behind chunk 1's prologue, not ahead of
                    # it; the first pair completes in this chunk.
                    emit_table()

                # ---------- consume this chunk's supergroups ----------
                for s in range(chunk_sg[c], chunk_sg[c + 1]):
                    consume_sg(s, lbv, c0, scv, c0)

    nc.compile()
    return nc


def kernel(**inputs):
    in_maps, tiles_g, gamma, node_of = _host_prep(inputs)

    key = (tiles_g, round(gamma, 6))
    if key not in _cache:
        _cache[key] = _build(tiles_g, gamma)
    nc = _cache[key]

    from concourse.bass_utils import run_bass_kernel_spmd

    res = run_bass_kernel_spmd(nc, in_maps, core_ids=list(range(NCORES)))

    out = np.empty((N, OUT_DIM), np.float32)
    for c in range(NCORES):
        colnode = node_of[c]
        valid = colnode >= 0
        out[c * NODES_PC + colnode[valid]] = \
            np.asarray(res.results[c]["outT"], np.float32)[:, valid].T
    return out
